# revision 49
# baseline (speedup 1.0000x reference)
"""Trainium2 Bass kernel for nn_CrossAttention (linear attention, elu+1 feature map).

Math (per batch element n of B=4, sequence L = V*HW = 20480, C=256, H=8 heads, d=32):
    qkv = xb @ W_qkv ; q,k,v splits
    phi(t) = elu(t)+1  (exactly max(t+1, min(exp(t), 1)))
    kv[h,m,d] = sum_l phi(k)[l,h,d] * v[l,h,m]
    z[l,h]   = 1 / (phi(q)[l,h,:] . sum_l phi(k)[l,h,:] + eps)
    y[l,h,m] = phi(q)[l,h,:] . kv[h,:,m] * z[l,h]
    out      = y @ W_proj + b_proj

Sharding: 8 cores = 4 batches x 2 L-halves (LH=10240 rows each); the only
cross-core traffic is a 67KB bf16 pair-AllReduce of the block-diag kv + ksum.

Final design (v10, HW-NTFF-profile driven; 385us baseline -> ~172us):
  * KX trick: accumulate KX = phi(k)^T x (x already stashed in SBUF) instead
    of phi(k)^T v: kills the v PSUM eviction AND halves the k/v projection
    (Wv is applied post-reduce to the tiny 256x256 KX).
  * all matmul operands bf16 (final rel err ~3.6e-3 << 2e-2 gate); PSUM f32.
  * phi = max(x+1, min(exp(x),1)) exactly: Act Exp (PSUM read), DVE
    tensor_scalar min-1 (bf16 SBUF, 2x mode), DVE scalar_tensor_tensor
    add/max (PSUM read).  gpsimd does only memsets/DMAs/collective — its
    tensor ops measured 7.4us per [128,512] call on HW.
  * phases: [k-proj + KX accumulation, software-pipelined, PE order
    proj(s), kvacc(s-2)] -> [pre-CC: PE-transpose KX, kv = KX@Wv, block-diag
    + ksum packed to 67KB] -> [bf16 pair AllReduce; the ENTIRE q^T
    projection phase (QLAG=40) runs during the CC's ~30us runtime dispatch
    latency + transfer] -> [Y: y/dn matmuls vs block-diag kv lhsT,
    z = reciprocal_approx_fast (the plain DVE reciprocal is 6 cycles/elem),
    y*z on DVE, out-proj, Act PSUM eviction, batched stores].
  * a tiny warmup AllReduce at t~10us absorbs part of the collective setup.
  * DMA dispatch (DIRECT2D) costs ~1.05us/instruction per hwdge queue: x
    loads are split into ramped tile sizes, first tiles on the gps SWDGE
    queue, output stores batched 2 groups per DMA.  Keeping bulk SWDGE
    traffic off the gps queue matters: it delays the collective dispatch.
"""

import os
import sys
import numpy as np

if "/opt/trn_rl_repo" not in sys.path:
    sys.path.insert(0, "/opt/trn_rl_repo")

# ---------------- problem constants (hardcoded per contest rules) -----------
BV, HW, C = 20, 4096, 256
NVIEW = 5
B = BV // NVIEW          # 4
H = 8
D = C // H               # 32
L = NVIEW * HW           # 20480
N_CORES = 8
LH = L // 2              # 10240 rows per core
EPS = 1e-6               # folded away: den >> 1e-6 always (phi>0, ksum~2e4)

NT = LH // 256           # 40 merged-loop groups (256 rows each)
QLAG = 40                # q^T projection trails by QLAG groups (CC cover)
DMA_RR = False           # keep x-loads off the gps queue: SWDGE traffic
                         # ahead of the collective delays its dispatch
OUT_BATCH = True         # 2 Y-groups per output store DMA
# xT stash tile col-ranges (first tiles small so the pipeline starts early)
XT_SPLIT = [512, 512, 1024] + [2048] * 4
XN_SPLIT = [2, 2, 4] + [8] * 9   # xN tile widths in 128-row l-tiles

_NC_CACHE = {}


def _build_nc(lh=LH, with_bias=False, collective=True, split_waits=True):
    """Build the Bass program (SPMD, one core's share: [C, lh] -> [lh, C])."""
    import concourse.bass as bass
    import concourse.mybir as mybir
    import concourse.tile as tile
    from contextlib import ExitStack

    f32 = mybir.dt.float32
    bf = mybir.dt.bfloat16
    AF = mybir.ActivationFunctionType
    OP = mybir.AluOpType
    PSUM = bass.MemorySpace.PSUM
    DRAM = bass.MemorySpace.DRAM

    assert lh % 512 == 0
    nt = lh // 256           # merged groups
    nq = nt // 2             # q^T pair-groups (512 l each)
    qlag = QLAG
    assert qlag % 2 == 0
    xt_off = [0]
    for w_ in XT_SPLIT:
        xt_off.append(xt_off[-1] + w_)
    assert xt_off[-1] == lh
    xn_off = [0]
    for w_ in XN_SPLIT:
        xn_off.append(xn_off[-1] + w_)
    assert xn_off[-1] == lh // 128
    nxt = len(XT_SPLIT)
    nxn = len(XN_SPLIT)

    nc = bass.Bass("TRN2", target_bir_lowering=False, debug=False,
                   num_devices=N_CORES)

    xT = nc.dram_tensor("xT", [C, lh], bf, kind="ExternalInput")
    xN = nc.dram_tensor("xN", [lh, 258], bf, kind="ExternalInput")
    wqkv = nc.dram_tensor("w_qkv", [C, 3 * C], bf, kind="ExternalInput")
    wproj = nc.dram_tensor("w_proj", [C, C], bf, kind="ExternalInput")
    bproj = nc.dram_tensor("b_proj", [1, C], f32, kind="ExternalInput")
    eye32 = nc.dram_tensor("eye32", [128, 128], bf, kind="ExternalInput")
    id128 = nc.dram_tensor("id128", [128, 128], bf, kind="ExternalInput")
    out = nc.dram_tensor("out", [lh, C], f32, kind="ExternalOutput")

    # out rows l = G*512 + j*128 + p  ->  [nt//2, 128, 4, 256]
    out_r = out[:].rearrange("(G j p) f -> G p j f", j=4, p=128)

    def xN_r(t):   # xN rows of stash tile t -> [128, XN_SPLIT[t], 258]
        lo, hi = 128 * xn_off[t], 128 * xn_off[t + 1]
        return xN[lo:hi, :].rearrange("(j p) f -> p j f", p=128)

    with tile.TileContext(nc) as tc, ExitStack() as ctx:
        const = ctx.enter_context(tc.tile_pool(name="const", bufs=1))
        stash = ctx.enter_context(tc.tile_pool(name="stash", bufs=1))
        sbw = ctx.enter_context(tc.tile_pool(name="sbw", bufs=4))
        dram = ctx.enter_context(tc.tile_pool(name="dram", bufs=1, space=DRAM))

        # ---- constants + x stashes -----------------------------------------
        # dispatch DMAs round-robin over BOTH hwdge queues (SP + Act): each
        # DIRECT2D dispatch costs ~1.05us on its sequencer, so one queue
        # serializes the whole load stream.
        def dma(dst, src):
            # x loads ride the idle gpsimd SWDGE queue; SP keeps the small
            # weight loads + stores.  (Act-queue DMAs delay Act compute.)
            q = nc.gpsimd if DMA_RR else nc.sync
            q.dma_start(dst, src)

        w_sb = [const.tile([128, 3 * C], bf, tag=f"w{h}", name=f"w{h}")
                for h in range(2)]
        wp_sb = [const.tile([128, C], bf, tag=f"wp{m}", name=f"wp{m}")
                 for m in range(2)]
        eye_sb = const.tile([128, 128], bf, tag="eye")
        xst = [[stash.tile([128, XT_SPLIT[t]], bf, tag=f"x{h}_{t}",
                           name=f"x{h}_{t}") for t in range(nxt)]
               for h in range(2)]
        xnst = [stash.tile([128, XN_SPLIT[t], 258], bf, tag=f"xn{t}",
                           name=f"xn{t}") for t in range(nxn)]

        def dma_xt(h, t):
            dma(xst[h][t][:],
                xT[128 * h:128 * (h + 1), xt_off[t]:xt_off[t + 1]])

        # first-needed first: x tile 0 (both c-halves) + xN tile 0 ride the
        # idle gps queue (its sequencer is free ~1.5us before SP), the small
        # weight loads go in parallel on SP
        for h in range(2):
            nc.gpsimd.dma_start(xst[h][0][:],
                                xT[128 * h:128 * (h + 1), 0:xt_off[1]])
        for h in range(2):
            nc.sync.dma_start(w_sb[h][:], wqkv[128 * h:128 * (h + 1), :])
        nc.gpsimd.dma_start(xnst[0][:], xN_r(0))
        dma(xnst[1][:], xN_r(1))
        nc.sync.dma_start(eye_sb[:], eye32[:, :])
        id_sb = const.tile([128, 128], bf, tag="id128")
        nc.sync.dma_start(id_sb[:], id128[:, :])
        for m in range(2):
            nc.sync.dma_start(wp_sb[m][:], wproj[128 * m:128 * (m + 1), :])
        if with_bias:
            brow_f = const.tile([1, C], f32, tag="brow_f")
            nc.sync.dma_start(brow_f[:], bproj[:, :])
            brow = const.tile([1, C], bf, tag="brow")
            nc.vector.tensor_copy(brow[:], brow_f[:])
            ones_k1 = const.tile([1, 128], bf, tag="ones_k1")
            nc.gpsimd.memset(ones_k1[:], 1.0)

        # prewarm the Exp act table while x streams in
        warm = const.tile([1, 8], bf, tag="warm")
        nc.scalar.activation(warm[:], eye_sb[0:1, 0:8], AF.Exp)

        # prewarm the collective path: the first CC pays ~35-40us of comm
        # setup in this runtime; burn it during the load phase
        if collective:
            ccw_in = dram.tile([1, 16], bf, tag="ccw_in")
            ccw_out = dram.tile([1, 16], bf, tag="ccw_out")
            nc.sync.dma_start(ccw_in[:], eye32[0:1, 0:16])
            nc.gpsimd.collective_compute(
                "AllReduce", mybir.AluOpType.add,
                replica_groups=[[2 * p, 2 * p + 1]
                                for p in range(N_CORES // 2)],
                ins=[ccw_in[:].opt()],
                outs=[ccw_out[:].opt()])

        done_xn = 2
        for t in range(1, nxt):
            for h in range(2):
                dma_xt(h, t)
            # keep xN deliveries paced with xT (both consumed in l-order)
            while done_xn < nxn and xn_off[done_xn] * 128 <= xt_off[t + 1] + 1024:
                dma(xnst[done_xn][:], xN_r(done_xn))
                done_xn += 1
        for t2 in range(done_xn, nxn):
            dma(xnst[t2][:], xN_r(t2))

        def _xt_slice(h, col, width):
            t = 0
            while xt_off[t + 1] <= col:
                t += 1
            o = col - xt_off[t]
            return xst[h][t][:, o:o + width]

        def xs128(h, lt):   # [c-half h, 128 l-cols] of l-tile lt
            return _xt_slice(h, 128 * lt, 128)

        def xs512(h, gg):   # [c-half h, 512 l-cols] of pair-group gg
            return _xt_slice(h, 512 * gg, 512)

        def xn128(lt):      # [128 l-part, 258] natural rows of l-tile lt
            t = 0
            while xn_off[t + 1] <= lt:
                t += 1
            return xnst[t][:, lt - xn_off[t], :]

        # ---- phi(q)^T stash: [nq] tiles of [128, 2, 512] bf16 --------------
        phq = [stash.tile([128, 2, 512], bf, tag=f"phq{g}", name=f"phq{g}")
               for g in range(nq)]

        # ================= merged loop: k proj + KX acc + q^T ===============
        with tc.tile_pool(name="ps_kv", bufs=1, space=PSUM) as ps_kv, \
             tc.tile_pool(name="ps_qt", bufs=2, space=PSUM) as ps_qt:
            ps_k_cm = tc.tile_pool(name="ps_k", bufs=2, space=PSUM)
            ps_k = ps_k_cm.__enter__()
            kvp = [ps_kv.tile([128, 258], f32, tag=f"kv{m}", name=f"kvp{m}")
                   for m in range(2)]

            live_kv = {}     # g -> phik
            live_q = {}      # gg -> qt_ps

            def emit_proj(g):
                k_ps = ps_k.tile([128, 2, 256], f32, tag="k")
                for j in range(2):
                    for h in range(2):
                        nc.tensor.matmul(
                            k_ps[:, j, :],
                            xs128(h, 2 * g + j),
                            w_sb[h][:, C:2 * C],
                            start=(h == 0), stop=(h == 1))
                return k_ps

            def emit_kchain(g, k_ps):
                # phi(x) = elu(x)+1 = max(x+1, min(exp(x), 1))  [exact]
                e_k = sbw.tile([128, 2, 256], bf, tag="e_k")
                e2_k = sbw.tile([128, 2, 256], bf, tag="e2_k")
                phik = sbw.tile([128, 2, 256], bf, tag="phik")
                nc.scalar.activation(e_k[:], k_ps[:], AF.Exp)
                nc.vector.tensor_scalar(e2_k[:], e_k[:], 1.0, None, op0=OP.min)
                nc.vector.scalar_tensor_tensor(phik[:], k_ps[:], 1.0, e2_k[:],
                                               op0=OP.add, op1=OP.max)
                live_kv[g] = phik

            def emit_qt(gg, pool):
                qt_ps = pool.tile([128, 2, 512], f32, tag="qt")
                for m in range(2):
                    for h in range(2):
                        nc.tensor.matmul(
                            qt_ps[:, m, :],
                            w_sb[h][:, 128 * m:128 * (m + 1)],
                            xs512(h, gg),
                            start=(h == 0), stop=(h == 1))
                live_q[gg] = qt_ps

            def emit_qchain(gg):
                qt_ps = live_q.pop(gg)
                e_q = sbw.tile([128, 2, 512], bf, tag="e_q")
                e2_q = sbw.tile([128, 2, 512], bf, tag="e2_q")
                nc.scalar.activation(e_q[:], qt_ps[:], AF.Exp)
                nc.vector.tensor_scalar(e2_q[:], e_q[:], 1.0, None, op0=OP.min)
                nc.vector.scalar_tensor_tensor(phq[gg][:], qt_ps[:], 1.0,
                                               e2_q[:], op0=OP.add, op1=OP.max)

            def emit_kvacc(g):
                phik = live_kv.pop(g)
                for j in range(2):
                    for m in range(2):
                        nc.tensor.matmul(
                            kvp[m][:, :],
                            phik[:, j, 128 * m:128 * (m + 1)],
                            xn128(2 * g + j),
                            start=(g == 0 and j == 0),
                            stop=(g == nt - 1 and j == 1),
                            skip_group_check=True)

            nq_in = (nt - qlag) // 2      # q pair-groups emitted in-loop
            for s in range(nt + 2):
                if s < nt:
                    k_ps = emit_proj(s)
                    emit_kchain(s, k_ps)
                if s >= qlag and (s - qlag) % 2 == 0 and (s - qlag) // 2 < nq_in:
                    gg = (s - qlag) // 2
                    emit_qt(gg, ps_qt)
                    emit_qchain(gg)
                if s >= 2:
                    emit_kvacc(s - 2)

            # ---- close the k-proj pool (frees 2 PSUM banks for the
            # transpose/kv scratch), evict KX partials (bf16) ----------------
            ps_k_cm.__exit__(None, None, None)
            kvev = [sbw.tile([128, 258], bf, tag=f"kvev{m}", name=f"kvev{m}")
                    for m in range(2)]
            for m in range(2):
                nc.vector.tensor_copy(kvev[m][:], kvp[m][:])

            with tc.tile_pool(name="ps_tx", bufs=1, space=PSUM) as ps_tx:
                # ---- pre-CC: kv_part = KX_part @ Wv, block-diag + ksum pack.
                # Exchanging the packed [2,128,130] bf16 (67KB) instead of raw
                # KX halves the CC transfer and makes the post-CC path trivial.
                kxt_ps = ps_tx.tile([128, 4, 128], bf, tag="kxt", name="kxt")
                for m in range(2):
                    for ch in range(2):
                        nc.tensor.transpose(
                            kxt_ps[:, 2 * m + ch, :],
                            kvev[m][:, 128 * ch:128 * (ch + 1)], id_sb[:])
                kxt_sb = sbw.tile([128, 4, 128], bf, tag="kxt_sb",
                                  name="kxt_sb")
                nc.vector.tensor_copy(kxt_sb[:].opt(), kxt_ps[:].opt())
                kvf = ps_tx.tile([128, 2, 256], f32, tag="kvf", name="kvf")
                for m in range(2):
                    for ch in range(2):
                        nc.tensor.matmul(kvf[:, m, :], kxt_sb[:, 2 * m + ch, :],
                                         w_sb[ch][:, 2 * C:3 * C],
                                         start=(ch == 0), stop=(ch == 1))
                kvpack = [sbw.tile([128, 130], bf, tag=f"kvpack{m}",
                                   name=f"kvpack{m}") for m in range(2)]
                for m in range(2):
                    nc.vector.memset(kvpack[m][:], 0.0)
                for m in range(2):
                    for hh in range(4):
                        nc.vector.tensor_copy(
                            kvpack[m][32 * hh:32 * (hh + 1),
                                      32 * hh:32 * (hh + 1)],
                            kvf[32 * hh:32 * (hh + 1), m,
                                128 * m + 32 * hh:128 * m + 32 * (hh + 1)])
                    nc.vector.tensor_copy(kvpack[m][:, 128:129],
                                          kvev[m][:, 256:257])

                # ---- cross-core AllReduce (pairs, bf16) --------------------
                kvb_in = dram.tile([2, 128, 130], bf, tag="kvb_in")
                kvb_out = dram.tile([2, 128, 130], bf, tag="kvb_out")
                for m in range(2):
                    nc.sync.dma_start(kvb_in[m], kvpack[m][:])
                if collective:
                    nc.gpsimd.collective_compute(
                        "AllReduce", mybir.AluOpType.add,
                        replica_groups=[[2 * p, 2 * p + 1]
                                        for p in range(N_CORES // 2)],
                        ins=[kvb_in[:].opt()],
                        outs=[kvb_out[:].opt()])
                else:  # single-core timeline-sim variant
                    nc.sync.dma_start(kvb_out[:], kvb_in[:])

                # q^T tail runs while the collective is in flight
                for gg in range(nq_in, nq):
                    emit_qt(gg, ps_qt)
                    emit_qchain(gg)

                # ---- post-CC: summed block-diag kv + ksum come back ready --
                kvr_sb = [sbw.tile([128, 130], bf, tag=f"kvr{m}",
                                   name=f"kvr{m}") for m in range(2)]
                for m in range(2):
                    nc.sync.dma_start(kvr_sb[m][:], kvb_out[m])
                kvblk = [kvr_sb[m][:, 0:128] for m in range(2)]
                ksum_f = [sbw.tile([128, 1], f32, tag=f"ksumf{m}",
                                   name=f"ksumf{m}") for m in range(2)]
                ksx = [const.tile([128, 128], bf, tag=f"ksx{m}",
                                  name=f"ksx{m}") for m in range(2)]
                for m in range(2):
                    nc.vector.tensor_copy(ksum_f[m][:], kvr_sb[m][:, 128:129])
                    nc.vector.tensor_scalar(
                        ksx[m][:], eye_sb[:], ksum_f[m][:], None, op0=OP.mult)

        # ================= Y: y/dn matmuls, z-scale, out proj, store ========
        with tc.tile_pool(name="ps_y", bufs=2, space=PSUM) as ps_y, \
             tc.tile_pool(name="ps_dn", bufs=2, space=PSUM) as ps_dn, \
             tc.tile_pool(name="ps_out", bufs=2, space=PSUM) as ps_out, \
             tc.tile_pool(name="sb2", bufs=3) as sb2:
            live_y = {}      # c -> y_sc

            def phq_slice(c, m):
                return phq[c // 2][:, m, 256 * (c % 2):256 * (c % 2) + 256]

            def emit_ydn(c):
                y_ps = ps_y.tile([128, 2, 256], f32, tag="y")
                dn_ps = ps_dn.tile([128, 2, 256], f32, tag="dn")
                for m in range(2):
                    nc.tensor.matmul(y_ps[:, m, :], kvblk[m],
                                     phq_slice(c, m), start=True, stop=True)
                for m in range(2):
                    nc.tensor.matmul(dn_ps[:, m, :], ksx[m][:],
                                     phq_slice(c, m), start=True, stop=True)
                z = sb2.tile([128, 2, 256], f32, tag="z")
                y_sc = sb2.tile([128, 2, 256], bf, tag="y_sc")
                nc.vector.reciprocal_approx_fast(z[:].opt(), dn_ps[:].opt())
                nc.vector.tensor_tensor(y_sc[:].opt(), y_ps[:].opt(),
                                        z[:].opt(), op=OP.mult)
                live_y[c] = y_sc

            live_ob = {}

            def emit_out(c):
                # two 256-row groups share one out_sb tile / one store DMA
                # (each DIRECT2D dispatch costs ~1.05us of SP-seq)
                y_sc = live_y.pop(c)
                out_ps = ps_out.tile([128, 2, 256], f32, tag="op")
                for j in range(2):
                    for m in range(2):
                        nc.tensor.matmul(
                            out_ps[:, j, :],
                            y_sc[:, m, 128 * j:128 * (j + 1)],
                            wp_sb[m][:],
                            start=(m == 0),
                            stop=(m == 1 and not with_bias))
                    if with_bias:
                        nc.tensor.matmul(out_ps[:, j, :], ones_k1[:], brow[:],
                                         start=False, stop=True)
                if not OUT_BATCH:
                    out_sb = sb2.tile([128, 2, 256], f32, tag="out_sb",
                                      name="out_sb")
                    nc.scalar.activation(out_sb[:].opt(), out_ps[:].opt(),
                                         AF.Copy)
                    nc.sync.dma_start(
                        out_r[c // 2][:, 2 * (c % 2):2 * (c % 2) + 2, :],
                        out_sb[:])
                    return
                if c % 2 == 0:
                    live_ob[c // 2] = sb2.tile([128, 4, 256], f32,
                                               tag="out_sb", name="out_sb")
                out_sb = live_ob[c // 2]
                half = out_sb[:, 2 * (c % 2):2 * (c % 2) + 2, :]
                nc.scalar.activation(half.opt(), out_ps[:].opt(), AF.Copy)
                if c % 2 == 1:
                    nc.sync.dma_start(out_r[c // 2], live_ob.pop(c // 2)[:])

            for c in range(nt + 1):
                if c < nt:
                    emit_ydn(c)
                if c >= 1:
                    emit_out(c - 1)

    # populate .instr bytes for InstISA subclasses (custom DVE ops) — raw
    # Bass skips this pass; without it walrus fails with "ISA wrong length"
    mybir.codegen_inst_isa_subclasses(nc)
    if split_waits:
        _split_multiwaits(nc)
    return nc


def _split_multiwaits(nc, limit=1):
    """This container's walrus rejects instructions carrying more than a
    couple of sync waits (CoreV3 setupSyncWait: 'Too many sync wait
    commands'). Splitting extra waits onto preceding same-engine NoOps is
    semantically identical on an in-order engine."""
    from concourse import mybir

    f = nc.m.functions[0]
    for b in f.blocks:
        new_insts = []
        for inst in b.instructions:
            si = getattr(inst, "sync_info", None)
            waits = list(si.on_wait) if (si and si.on_wait) else []
            if len(waits) > limit:
                head, keep = waits[:-limit], waits[-limit:]
                for w0 in range(0, len(head), limit):
                    nop = mybir.InstNoOp(
                        name=nc.get_next_instruction_name(), ins=[], outs=[])
                    nop.engine = inst.engine
                    nop.sync_info = mybir.SyncInfo(
                        on_wait=head[w0:w0 + limit], on_update=[])
                    new_insts.append(nop)
                inst.sync_info = mybir.SyncInfo(
                    on_wait=keep, on_update=list(si.on_update or []))
            new_insts.append(inst)
        b.instructions[:] = new_insts


def _build_null_nc(lh=LH):
    """Minimal program with the same I/O signature (for dispatch-overhead
    measurement in test.py)."""
    import concourse.bass as bass
    import concourse.mybir as mybir
    import concourse.tile as tile

    f32 = mybir.dt.float32
    bf = mybir.dt.bfloat16
    nc = bass.Bass("TRN2", target_bir_lowering=False, debug=False,
                   num_devices=N_CORES)
    xT = nc.dram_tensor("xT", [C, lh], bf, kind="ExternalInput")
    nc.dram_tensor("xN", [lh, 258], bf, kind="ExternalInput")
    nc.dram_tensor("w_qkv", [C, 3 * C], bf, kind="ExternalInput")
    nc.dram_tensor("w_proj", [C, C], bf, kind="ExternalInput")
    nc.dram_tensor("b_proj", [1, C], f32, kind="ExternalInput")
    nc.dram_tensor("eye32", [128, 128], bf, kind="ExternalInput")
    nc.dram_tensor("id128", [128, 128], bf, kind="ExternalInput")
    out = nc.dram_tensor("out", [lh, C], f32, kind="ExternalOutput")
    with tile.TileContext(nc) as tc:
        with tc.tile_pool(name="p", bufs=1) as p:
            t = p.tile([1, 256], bf, tag="t", name="t")
            nc.sync.dma_start(t[:], xT[0:1, 0:256])
            nc.sync.dma_start(out[0:1, :], t[:].bitcast(mybir.dt.uint16))
    _split_multiwaits(nc)
    return nc


class _Runner:
    """Cached jit(shard_map(bass_exec)) over the 8 axon trn2 cores."""

    def __init__(self, nc):
        import jax
        import jax.numpy as jnp
        from jax.sharding import Mesh, PartitionSpec
        from jax.experimental.shard_map import shard_map
        import concourse.mybir as mybir
        from concourse import bass2jax

        bass2jax.install_neuronx_cc_hook()
        self.jax, self.jnp = jax, jnp

        partition_name = (nc.partition_id_tensor.name
                          if nc.partition_id_tensor else None)
        in_names, out_names, out_avals = [], [], []
        for alloc in nc.m.functions[0].allocations:
            if not isinstance(alloc, mybir.MemoryLocationSet):
                continue
            name = alloc.memorylocations[0].name
            if alloc.kind == "ExternalInput":
                if name != partition_name:
                    in_names.append(name)
            elif alloc.kind == "ExternalOutput":
                out_names.append(name)
                out_avals.append(jax.core.ShapedArray(
                    tuple(alloc.tensor_shape), mybir.dt.np(alloc.dtype)))
        assert nc.dbg_addr is None
        self.in_names, self.out_names, self.out_avals = in_names, out_names, out_avals
        n_params = len(in_names)
        all_in_names = in_names + out_names
        if partition_name is not None:
            all_in_names = all_in_names + [partition_name]
        all_in_names = tuple(all_in_names)

        def _body(*args):
            operands = list(args)
            if partition_name is not None:
                operands.append(bass2jax.partition_id_tensor())
            outs = bass2jax._bass_exec_p.bind(
                *operands,
                out_avals=tuple(out_avals),
                in_names=all_in_names,
                out_names=tuple(out_names),
                lowering_input_output_aliases=(),
                sim_require_finite=True,
                sim_require_nnan=True,
                nc=nc,
            )
            return tuple(outs)

        devices = jax.devices()[:N_CORES]
        self.mesh = Mesh(np.asarray(devices), ("core",))
        spec = PartitionSpec("core")
        n_outs = len(out_names)
        self.donate = tuple(range(n_params, n_params + n_outs))
        self.fn = jax.jit(
            shard_map(_body, mesh=self.mesh, in_specs=(spec,) * (n_params + n_outs),
                      out_specs=(spec,) * n_outs, check_rep=False),
            donate_argnums=self.donate, keep_unused=True)
        self.sharding = jax.sharding.NamedSharding(self.mesh, spec)

        def _zeros():
            return tuple(
                jnp.zeros((N_CORES * a.shape[0], *a.shape[1:]), a.dtype)
                for a in out_avals)
        self.zeros_fn = jax.jit(_zeros, out_shardings=(self.sharding,) * n_outs)

    def place_inputs(self, in_maps):
        concat = [np.concatenate([np.asarray(m[n]) for m in in_maps], axis=0)
                  for n in self.in_names]
        return [self.jax.device_put(a, self.sharding) for a in concat]

    def call(self, dev_in):
        outs = self.fn(*dev_in, *self.zeros_fn())
        self.jax.block_until_ready(outs)
        return outs

    def run(self, in_maps):
        outs = self.call(self.place_inputs(in_maps))
        res = []
        for c in range(N_CORES):
            res.append({n: np.asarray(outs[i]).reshape(
                N_CORES, *self.out_avals[i].shape)[c]
                for i, n in enumerate(self.out_names)})
        return res


def _get_runner(lh=LH, with_bias=False, null=False):
    key = (lh, with_bias, null)
    if key not in _NC_CACHE:
        nc = _build_null_nc(lh) if null else _build_nc(lh, with_bias)
        _NC_CACHE[key] = _Runner(nc)
    return _NC_CACHE[key]


def _bf16(a):
    import ml_dtypes
    return np.asarray(a, dtype=ml_dtypes.bfloat16)


def _make_eye32():
    return np.kron(np.eye(4, dtype=np.float32), np.ones((32, 32), np.float32))


def _make_in_maps(x, W_qkv, W_proj, b_proj, lh=LH):
    import ml_dtypes
    ncores_b = B * (L // lh)
    xb = np.ascontiguousarray(x.reshape(B, L // lh, lh, C))
    eye = _bf16(_make_eye32())
    ident = _bf16(np.eye(128, dtype=np.float32))
    w = _bf16(W_qkv)
    wp = _bf16(W_proj)
    bp = np.ascontiguousarray(b_proj, np.float32).reshape(1, C)
    in_maps = []
    for c in range(ncores_b):
        bb, hh = divmod(c, L // lh)
        xc = xb[bb, hh]                                   # [lh, C] f32
        xTc = _bf16(np.ascontiguousarray(xc.T))           # [C, lh]
        xNc = np.ones((lh, 258), dtype=ml_dtypes.bfloat16)
        xNc[:, 0:C] = _bf16(xc)
        in_maps.append({"xT": xTc, "xN": xNc, "w_qkv": w, "w_proj": wp,
                        "b_proj": bp, "eye32": eye, "id128": ident})
    return in_maps


def _assemble(results):
    outs = [results[c]["out"] for c in range(N_CORES)]
    y = np.stack(outs).reshape(B, 2, LH, C).reshape(B, L, C)
    return np.ascontiguousarray(y.reshape(BV, HW, C), dtype=np.float32)


def _run(x, W_qkv, W_proj, b_proj):
    with_bias = bool(np.any(b_proj))
    runner = _get_runner(LH, with_bias)
    in_maps = _make_in_maps(x, W_qkv, W_proj, b_proj)
    return _assemble(runner.run(in_maps))


def kernel(x, W_qkv, W_proj, b_proj):
    return _run(np.asarray(x, np.float32), np.asarray(W_qkv, np.float32),
                np.asarray(W_proj, np.float32), np.asarray(b_proj, np.float32))


# revision 50
# speedup vs baseline: 1.0259x; 1.0259x over previous
"""Trainium2 Bass kernel for nn_CrossAttention (linear attention, elu+1 feature map).

Math (per batch element n of B=4, sequence L = V*HW = 20480, C=256, H=8 heads, d=32):
    qkv = xb @ W_qkv ; q,k,v splits
    phi(t) = elu(t)+1  (exactly max(t+1, min(exp(t), 1)))
    kv[h,m,d] = sum_l phi(k)[l,h,d] * v[l,h,m]
    z[l,h]   = 1 / (phi(q)[l,h,:] . sum_l phi(k)[l,h,:] + eps)
    y[l,h,m] = phi(q)[l,h,:] . kv[h,:,m] * z[l,h]
    out      = y @ W_proj + b_proj

Sharding: 8 cores = 4 batches x 2 L-halves (LH=10240 rows each); the only
cross-core traffic is a 67KB bf16 pair-AllReduce of the block-diag kv + ksum.

Final design (v10, HW-NTFF-profile driven; 385us baseline -> ~172us):
  * KX trick: accumulate KX = phi(k)^T x (x already stashed in SBUF) instead
    of phi(k)^T v: kills the v PSUM eviction AND halves the k/v projection
    (Wv is applied post-reduce to the tiny 256x256 KX).
  * all matmul operands bf16 (final rel err ~3.6e-3 << 2e-2 gate); PSUM f32.
  * phi = max(x+1, min(exp(x),1)) exactly: Act Exp (PSUM read), DVE
    tensor_scalar min-1 (bf16 SBUF, 2x mode), DVE scalar_tensor_tensor
    add/max (PSUM read).  gpsimd does only memsets/DMAs/collective — its
    tensor ops measured 7.4us per [128,512] call on HW.
  * phases: [k-proj + KX accumulation, software-pipelined, PE order
    proj(s), kvacc(s-2)] -> [pre-CC: PE-transpose KX, kv = KX@Wv, block-diag
    + ksum packed to 67KB] -> [bf16 pair AllReduce; the ENTIRE q^T
    projection phase (QLAG=40) runs during the CC's ~30us runtime dispatch
    latency + transfer] -> [Y: y/dn matmuls vs block-diag kv lhsT,
    z = reciprocal_approx_fast (the plain DVE reciprocal is 6 cycles/elem),
    y*z on DVE, out-proj, Act PSUM eviction, batched stores].
  * a tiny warmup AllReduce at t~10us absorbs part of the collective setup.
  * DMA dispatch (DIRECT2D) costs ~1.05us/instruction per hwdge queue: x
    loads are split into ramped tile sizes, first tiles on the gps SWDGE
    queue, output stores batched 2 groups per DMA.  Keeping bulk SWDGE
    traffic off the gps queue matters: it delays the collective dispatch.
"""

import os
import sys
import numpy as np

if "/opt/trn_rl_repo" not in sys.path:
    sys.path.insert(0, "/opt/trn_rl_repo")

# ---------------- problem constants (hardcoded per contest rules) -----------
BV, HW, C = 20, 4096, 256
NVIEW = 5
B = BV // NVIEW          # 4
H = 8
D = C // H               # 32
L = NVIEW * HW           # 20480
N_CORES = 8
LH = L // 2              # 10240 rows per core
EPS = 1e-6               # folded away: den >> 1e-6 always (phi>0, ksum~2e4)

NT = LH // 256           # 40 merged-loop groups (256 rows each)
QLAG = 40                # q^T projection trails by QLAG groups (CC cover)
DMA_RR = False           # keep x-loads off the gps queue: SWDGE traffic
                         # ahead of the collective delays its dispatch
OUT_BATCH = True         # 2 Y-groups per output store DMA
# xT stash tile col-ranges (first tiles small so the pipeline starts early)
XT_SPLIT = [512, 512, 1024] + [2048] * 4
XN_SPLIT = [2, 2, 4] + [8] * 9   # xN tile widths in 128-row l-tiles

_NC_CACHE = {}


def _build_nc(lh=LH, with_bias=False, collective=True, split_waits=True):
    """Build the Bass program (SPMD, one core's share: [C, lh] -> [lh, C])."""
    import concourse.bass as bass
    import concourse.mybir as mybir
    import concourse.tile as tile
    from contextlib import ExitStack

    f32 = mybir.dt.float32
    bf = mybir.dt.bfloat16
    AF = mybir.ActivationFunctionType
    OP = mybir.AluOpType
    PSUM = bass.MemorySpace.PSUM
    DRAM = bass.MemorySpace.DRAM

    assert lh % 512 == 0
    nt = lh // 256           # merged groups
    nq = nt // 2             # q^T pair-groups (512 l each)
    qlag = QLAG
    assert qlag % 2 == 0
    xt_off = [0]
    for w_ in XT_SPLIT:
        xt_off.append(xt_off[-1] + w_)
    assert xt_off[-1] == lh
    xn_off = [0]
    for w_ in XN_SPLIT:
        xn_off.append(xn_off[-1] + w_)
    assert xn_off[-1] == lh // 128
    nxt = len(XT_SPLIT)
    nxn = len(XN_SPLIT)

    nc = bass.Bass("TRN2", target_bir_lowering=False, debug=False,
                   num_devices=N_CORES)

    xT = nc.dram_tensor("xT", [C, lh], bf, kind="ExternalInput")
    xN = nc.dram_tensor("xN", [lh, 258], bf, kind="ExternalInput")
    wqkv = nc.dram_tensor("w_qkv", [C, 3 * C], bf, kind="ExternalInput")
    wproj = nc.dram_tensor("w_proj", [C, C], bf, kind="ExternalInput")
    bproj = nc.dram_tensor("b_proj", [1, C], f32, kind="ExternalInput")
    eye32 = nc.dram_tensor("eye32", [128, 128], bf, kind="ExternalInput")
    id128 = nc.dram_tensor("id128", [128, 128], bf, kind="ExternalInput")
    out = nc.dram_tensor("out", [lh, C], f32, kind="ExternalOutput")

    # out rows l = G*512 + j*128 + p  ->  [nt//2, 128, 4, 256]
    out_r = out[:].rearrange("(G j p) f -> G p j f", j=4, p=128)

    def xN_r(t):   # xN rows of stash tile t -> [128, XN_SPLIT[t], 258]
        lo, hi = 128 * xn_off[t], 128 * xn_off[t + 1]
        return xN[lo:hi, :].rearrange("(j p) f -> p j f", p=128)

    with tile.TileContext(nc) as tc, ExitStack() as ctx:
        const = ctx.enter_context(tc.tile_pool(name="const", bufs=1))
        stash = ctx.enter_context(tc.tile_pool(name="stash", bufs=1))
        sbw = ctx.enter_context(tc.tile_pool(name="sbw", bufs=4))
        dram = ctx.enter_context(tc.tile_pool(name="dram", bufs=1, space=DRAM))

        # ---- constants + x stashes -----------------------------------------
        # dispatch DMAs round-robin over BOTH hwdge queues (SP + Act): each
        # DIRECT2D dispatch costs ~1.05us on its sequencer, so one queue
        # serializes the whole load stream.
        def dma(dst, src):
            # x loads ride the idle gpsimd SWDGE queue; SP keeps the small
            # weight loads + stores.  (Act-queue DMAs delay Act compute.)
            q = nc.gpsimd if DMA_RR else nc.sync
            q.dma_start(dst, src)

        w_sb = [const.tile([128, 3 * C], bf, tag=f"w{h}", name=f"w{h}")
                for h in range(2)]
        wp_sb = [const.tile([128, C], bf, tag=f"wp{m}", name=f"wp{m}")
                 for m in range(2)]
        eye_sb = const.tile([128, 128], bf, tag="eye")
        xst = [[stash.tile([128, XT_SPLIT[t]], bf, tag=f"x{h}_{t}",
                           name=f"x{h}_{t}") for t in range(nxt)]
               for h in range(2)]
        xnst = [stash.tile([128, XN_SPLIT[t], 258], bf, tag=f"xn{t}",
                           name=f"xn{t}") for t in range(nxn)]

        def dma_xt(h, t):
            dma(xst[h][t][:],
                xT[128 * h:128 * (h + 1), xt_off[t]:xt_off[t + 1]])

        # first-needed first: x tile 0 (both c-halves) + xN tile 0 ride the
        # idle gps queue (its sequencer is free ~1.5us before SP), the small
        # weight loads go in parallel on SP
        for h in range(2):
            nc.gpsimd.dma_start(xst[h][0][:],
                                xT[128 * h:128 * (h + 1), 0:xt_off[1]])
        for h in range(2):
            nc.sync.dma_start(w_sb[h][:], wqkv[128 * h:128 * (h + 1), :])
        nc.gpsimd.dma_start(xnst[0][:], xN_r(0))
        dma(xnst[1][:], xN_r(1))
        nc.sync.dma_start(eye_sb[:], eye32[:, :])
        id_sb = const.tile([128, 128], bf, tag="id128")
        nc.sync.dma_start(id_sb[:], id128[:, :])
        for m in range(2):
            nc.sync.dma_start(wp_sb[m][:], wproj[128 * m:128 * (m + 1), :])
        if with_bias:
            brow_f = const.tile([1, C], f32, tag="brow_f")
            nc.sync.dma_start(brow_f[:], bproj[:, :])
            brow = const.tile([1, C], bf, tag="brow")
            nc.vector.tensor_copy(brow[:], brow_f[:])
            ones_k1 = const.tile([1, 128], bf, tag="ones_k1")
            nc.gpsimd.memset(ones_k1[:], 1.0)

        # prewarm the Exp act table while x streams in
        warm = const.tile([1, 8], bf, tag="warm")
        nc.scalar.activation(warm[:], eye_sb[0:1, 0:8], AF.Exp)

        # prewarm the collective path: the first CC pays ~35-40us of comm
        # setup in this runtime; burn it during the load phase
        if collective:
            ccw_in = dram.tile([1, 16], bf, tag="ccw_in")
            ccw_out = dram.tile([1, 16], bf, tag="ccw_out")
            nc.sync.dma_start(ccw_in[:], eye32[0:1, 0:16])
            nc.gpsimd.collective_compute(
                "AllReduce", mybir.AluOpType.add,
                replica_groups=[[2 * p, 2 * p + 1]
                                for p in range(N_CORES // 2)],
                ins=[ccw_in[:].opt()],
                outs=[ccw_out[:].opt()])

        done_xn = 2
        for t in range(1, nxt):
            for h in range(2):
                dma_xt(h, t)
            # keep xN deliveries paced with xT (both consumed in l-order)
            while done_xn < nxn and xn_off[done_xn] * 128 <= xt_off[t + 1] + 1024:
                dma(xnst[done_xn][:], xN_r(done_xn))
                done_xn += 1
        for t2 in range(done_xn, nxn):
            dma(xnst[t2][:], xN_r(t2))

        def _xt_slice(h, col, width):
            t = 0
            while xt_off[t + 1] <= col:
                t += 1
            o = col - xt_off[t]
            return xst[h][t][:, o:o + width]

        def xs128(h, lt):   # [c-half h, 128 l-cols] of l-tile lt
            return _xt_slice(h, 128 * lt, 128)

        def xs512(h, gg):   # [c-half h, 512 l-cols] of pair-group gg
            return _xt_slice(h, 512 * gg, 512)

        def xn128(lt):      # [128 l-part, 258] natural rows of l-tile lt
            t = 0
            while xn_off[t + 1] <= lt:
                t += 1
            return xnst[t][:, lt - xn_off[t], :]

        # ---- phi(q)^T stash: [nq] tiles of [128, 2, 512] bf16 --------------
        phq = [stash.tile([128, 2, 512], bf, tag=f"phq{g}", name=f"phq{g}")
               for g in range(nq)]

        # ================= merged loop: k proj + KX acc + q^T ===============
        with tc.tile_pool(name="ps_kv", bufs=1, space=PSUM) as ps_kv, \
             tc.tile_pool(name="ps_qt", bufs=2, space=PSUM) as ps_qt:
            ps_k_cm = tc.tile_pool(name="ps_k", bufs=2, space=PSUM)
            ps_k = ps_k_cm.__enter__()
            kvp = [ps_kv.tile([128, 258], f32, tag=f"kv{m}", name=f"kvp{m}")
                   for m in range(2)]

            live_kv = {}     # g -> phik
            live_q = {}      # gg -> qt_ps

            def emit_proj(g):
                k_ps = ps_k.tile([128, 2, 256], f32, tag="k")
                for j in range(2):
                    for h in range(2):
                        nc.tensor.matmul(
                            k_ps[:, j, :],
                            xs128(h, 2 * g + j),
                            w_sb[h][:, C:2 * C],
                            start=(h == 0), stop=(h == 1))
                return k_ps

            def emit_kchain(g, k_ps):
                # phi(x) = elu(x)+1 = max(x+1, min(exp(x), 1))  [exact]
                e_k = sbw.tile([128, 2, 256], bf, tag="e_k")
                e2_k = sbw.tile([128, 2, 256], bf, tag="e2_k")
                phik = sbw.tile([128, 2, 256], bf, tag="phik")
                nc.scalar.activation(e_k[:], k_ps[:], AF.Exp)
                nc.vector.tensor_scalar(e2_k[:], e_k[:], 1.0, None, op0=OP.min)
                nc.vector.scalar_tensor_tensor(phik[:], k_ps[:], 1.0, e2_k[:],
                                               op0=OP.add, op1=OP.max)
                live_kv[g] = phik

            def emit_qt(gg, pool):
                qt_ps = pool.tile([128, 2, 512], f32, tag="qt")
                for m in range(2):
                    for h in range(2):
                        nc.tensor.matmul(
                            qt_ps[:, m, :],
                            w_sb[h][:, 128 * m:128 * (m + 1)],
                            xs512(h, gg),
                            start=(h == 0), stop=(h == 1))
                live_q[gg] = qt_ps

            def emit_qchain(gg):
                qt_ps = live_q.pop(gg)
                e_q = sbw.tile([128, 2, 512], bf, tag="e_q")
                e2_q = sbw.tile([128, 2, 512], bf, tag="e2_q")
                nc.scalar.activation(e_q[:], qt_ps[:], AF.Exp)
                nc.vector.tensor_scalar(e2_q[:], e_q[:], 1.0, None, op0=OP.min)
                nc.vector.scalar_tensor_tensor(phq[gg][:], qt_ps[:], 1.0,
                                               e2_q[:], op0=OP.add, op1=OP.max)

            def emit_kvacc(g):
                phik = live_kv.pop(g)
                for j in range(2):
                    for m in range(2):
                        nc.tensor.matmul(
                            kvp[m][:, :],
                            phik[:, j, 128 * m:128 * (m + 1)],
                            xn128(2 * g + j),
                            start=(g == 0 and j == 0),
                            stop=(g == nt - 1 and j == 1),
                            skip_group_check=True)

            nq_in = (nt - qlag) // 2      # q pair-groups emitted in-loop
            for s in range(nt + 2):
                if s < nt:
                    k_ps = emit_proj(s)
                    emit_kchain(s, k_ps)
                if s >= qlag and (s - qlag) % 2 == 0 and (s - qlag) // 2 < nq_in:
                    gg = (s - qlag) // 2
                    emit_qt(gg, ps_qt)
                    emit_qchain(gg)
                if s >= 2:
                    emit_kvacc(s - 2)

            # ---- close the k-proj pool (frees 2 PSUM banks for the
            # transpose/kv scratch), evict KX partials (bf16) ----------------
            ps_k_cm.__exit__(None, None, None)
            kvev = [sbw.tile([128, 258], bf, tag=f"kvev{m}", name=f"kvev{m}")
                    for m in range(2)]
            for m in range(2):
                nc.vector.tensor_copy(kvev[m][:], kvp[m][:])

            with tc.tile_pool(name="ps_tx", bufs=1, space=PSUM) as ps_tx:
                # ---- pre-CC: kv_part = KX_part @ Wv, block-diag + ksum pack.
                # Exchanging the packed [2,128,130] bf16 (67KB) instead of raw
                # KX halves the CC transfer and makes the post-CC path trivial.
                kxt_ps = ps_tx.tile([128, 4, 128], bf, tag="kxt", name="kxt")
                for m in range(2):
                    for ch in range(2):
                        nc.tensor.transpose(
                            kxt_ps[:, 2 * m + ch, :],
                            kvev[m][:, 128 * ch:128 * (ch + 1)], id_sb[:])
                kxt_sb = sbw.tile([128, 4, 128], bf, tag="kxt_sb",
                                  name="kxt_sb")
                nc.vector.tensor_copy(kxt_sb[:].opt(), kxt_ps[:].opt())
                kvf = ps_tx.tile([128, 2, 256], f32, tag="kvf", name="kvf")
                for m in range(2):
                    for ch in range(2):
                        nc.tensor.matmul(kvf[:, m, :], kxt_sb[:, 2 * m + ch, :],
                                         w_sb[ch][:, 2 * C:3 * C],
                                         start=(ch == 0), stop=(ch == 1))
                kvpack = sbw.tile([128, 2, 130], bf, tag="kvpack",
                                  name="kvpack")
                nc.vector.memset(kvpack[:], 0.0)
                for m in range(2):
                    for hh in range(4):
                        nc.vector.tensor_copy(
                            kvpack[32 * hh:32 * (hh + 1), m,
                                   32 * hh:32 * (hh + 1)],
                            kvf[32 * hh:32 * (hh + 1), m,
                                128 * m + 32 * hh:128 * m + 32 * (hh + 1)])
                    nc.vector.tensor_copy(kvpack[:, m, 128:129],
                                          kvev[m][:, 256:257])

                # ---- cross-core AllReduce (pairs, bf16) --------------------
                # partition-major [128, 2, 130] layout: ONE pack DMA out and
                # ONE readback DMA (contiguous 520B/partition descriptors)
                # instead of four 260B-row stragglers around the collective
                kvb_in = dram.tile([128, 2, 130], bf, tag="kvb_in")
                kvb_out = dram.tile([128, 2, 130], bf, tag="kvb_out")
                nc.sync.dma_start(kvb_in[:, :, :], kvpack[:])
                if collective:
                    nc.gpsimd.collective_compute(
                        "AllReduce", mybir.AluOpType.add,
                        replica_groups=[[2 * p, 2 * p + 1]
                                        for p in range(N_CORES // 2)],
                        ins=[kvb_in[:].opt()],
                        outs=[kvb_out[:].opt()])
                else:  # single-core timeline-sim variant
                    nc.sync.dma_start(kvb_out[:], kvb_in[:])

                # q^T tail runs while the collective is in flight
                for gg in range(nq_in, nq):
                    emit_qt(gg, ps_qt)
                    emit_qchain(gg)

                # ---- post-CC: summed block-diag kv + ksum come back ready --
                kvr_sb = sbw.tile([128, 2, 130], bf, tag="kvr", name="kvr")
                nc.sync.dma_start(kvr_sb[:], kvb_out[:, :, :])
                kvblk = [kvr_sb[:, m, 0:128] for m in range(2)]
                ksum_f = [sbw.tile([128, 1], f32, tag=f"ksumf{m}",
                                   name=f"ksumf{m}") for m in range(2)]
                ksx = [const.tile([128, 128], bf, tag=f"ksx{m}",
                                  name=f"ksx{m}") for m in range(2)]
                for m in range(2):
                    nc.vector.tensor_copy(ksum_f[m][:], kvr_sb[:, m, 128:129])
                    nc.vector.tensor_scalar(
                        ksx[m][:], eye_sb[:], ksum_f[m][:], None, op0=OP.mult)

        # ================= Y: y/dn matmuls, z-scale, out proj, store ========
        with tc.tile_pool(name="ps_y", bufs=2, space=PSUM) as ps_y, \
             tc.tile_pool(name="ps_dn", bufs=2, space=PSUM) as ps_dn, \
             tc.tile_pool(name="ps_out", bufs=2, space=PSUM) as ps_out, \
             tc.tile_pool(name="sb2", bufs=3) as sb2:
            live_y = {}      # c -> y_sc

            def phq_slice(c, m):
                return phq[c // 2][:, m, 256 * (c % 2):256 * (c % 2) + 256]

            def emit_ydn(c):
                y_ps = ps_y.tile([128, 2, 256], f32, tag="y")
                dn_ps = ps_dn.tile([128, 2, 256], f32, tag="dn")
                for m in range(2):
                    nc.tensor.matmul(y_ps[:, m, :], kvblk[m],
                                     phq_slice(c, m), start=True, stop=True)
                for m in range(2):
                    nc.tensor.matmul(dn_ps[:, m, :], ksx[m][:],
                                     phq_slice(c, m), start=True, stop=True)
                z = sb2.tile([128, 2, 256], f32, tag="z")
                y_sc = sb2.tile([128, 2, 256], bf, tag="y_sc")
                nc.vector.reciprocal_approx_fast(z[:].opt(), dn_ps[:].opt())
                nc.vector.tensor_tensor(y_sc[:].opt(), y_ps[:].opt(),
                                        z[:].opt(), op=OP.mult)
                live_y[c] = y_sc

            live_ob = {}

            def emit_out(c):
                # two 256-row groups share one out_sb tile / one store DMA
                # (each DIRECT2D dispatch costs ~1.05us of SP-seq)
                y_sc = live_y.pop(c)
                out_ps = ps_out.tile([128, 2, 256], f32, tag="op")
                for j in range(2):
                    for m in range(2):
                        nc.tensor.matmul(
                            out_ps[:, j, :],
                            y_sc[:, m, 128 * j:128 * (j + 1)],
                            wp_sb[m][:],
                            start=(m == 0),
                            stop=(m == 1 and not with_bias))
                    if with_bias:
                        nc.tensor.matmul(out_ps[:, j, :], ones_k1[:], brow[:],
                                         start=False, stop=True)
                if not OUT_BATCH:
                    out_sb = sb2.tile([128, 2, 256], f32, tag="out_sb",
                                      name="out_sb")
                    nc.scalar.activation(out_sb[:].opt(), out_ps[:].opt(),
                                         AF.Copy)
                    nc.sync.dma_start(
                        out_r[c // 2][:, 2 * (c % 2):2 * (c % 2) + 2, :],
                        out_sb[:])
                    return
                if c % 2 == 0:
                    live_ob[c // 2] = sb2.tile([128, 4, 256], f32,
                                               tag="out_sb", name="out_sb")
                out_sb = live_ob[c // 2]
                half = out_sb[:, 2 * (c % 2):2 * (c % 2) + 2, :]
                nc.scalar.activation(half.opt(), out_ps[:].opt(), AF.Copy)
                if c % 2 == 1:
                    nc.sync.dma_start(out_r[c // 2], live_ob.pop(c // 2)[:])

            for c in range(nt + 1):
                if c < nt:
                    emit_ydn(c)
                if c >= 1:
                    emit_out(c - 1)

    # populate .instr bytes for InstISA subclasses (custom DVE ops) — raw
    # Bass skips this pass; without it walrus fails with "ISA wrong length"
    mybir.codegen_inst_isa_subclasses(nc)
    if split_waits:
        _split_multiwaits(nc)
    return nc


def _split_multiwaits(nc, limit=1):
    """This container's walrus rejects instructions carrying more than a
    couple of sync waits (CoreV3 setupSyncWait: 'Too many sync wait
    commands'). Splitting extra waits onto preceding same-engine NoOps is
    semantically identical on an in-order engine."""
    from concourse import mybir

    f = nc.m.functions[0]
    for b in f.blocks:
        new_insts = []
        for inst in b.instructions:
            si = getattr(inst, "sync_info", None)
            waits = list(si.on_wait) if (si and si.on_wait) else []
            if len(waits) > limit:
                head, keep = waits[:-limit], waits[-limit:]
                for w0 in range(0, len(head), limit):
                    nop = mybir.InstNoOp(
                        name=nc.get_next_instruction_name(), ins=[], outs=[])
                    nop.engine = inst.engine
                    nop.sync_info = mybir.SyncInfo(
                        on_wait=head[w0:w0 + limit], on_update=[])
                    new_insts.append(nop)
                inst.sync_info = mybir.SyncInfo(
                    on_wait=keep, on_update=list(si.on_update or []))
            new_insts.append(inst)
        b.instructions[:] = new_insts


def _build_null_nc(lh=LH):
    """Minimal program with the same I/O signature (for dispatch-overhead
    measurement in test.py)."""
    import concourse.bass as bass
    import concourse.mybir as mybir
    import concourse.tile as tile

    f32 = mybir.dt.float32
    bf = mybir.dt.bfloat16
    nc = bass.Bass("TRN2", target_bir_lowering=False, debug=False,
                   num_devices=N_CORES)
    xT = nc.dram_tensor("xT", [C, lh], bf, kind="ExternalInput")
    nc.dram_tensor("xN", [lh, 258], bf, kind="ExternalInput")
    nc.dram_tensor("w_qkv", [C, 3 * C], bf, kind="ExternalInput")
    nc.dram_tensor("w_proj", [C, C], bf, kind="ExternalInput")
    nc.dram_tensor("b_proj", [1, C], f32, kind="ExternalInput")
    nc.dram_tensor("eye32", [128, 128], bf, kind="ExternalInput")
    nc.dram_tensor("id128", [128, 128], bf, kind="ExternalInput")
    out = nc.dram_tensor("out", [lh, C], f32, kind="ExternalOutput")
    with tile.TileContext(nc) as tc:
        with tc.tile_pool(name="p", bufs=1) as p:
            t = p.tile([1, 256], bf, tag="t", name="t")
            nc.sync.dma_start(t[:], xT[0:1, 0:256])
            nc.sync.dma_start(out[0:1, :], t[:].bitcast(mybir.dt.uint16))
    _split_multiwaits(nc)
    return nc


class _Runner:
    """Cached jit(shard_map(bass_exec)) over the 8 axon trn2 cores."""

    def __init__(self, nc):
        import jax
        import jax.numpy as jnp
        from jax.sharding import Mesh, PartitionSpec
        from jax.experimental.shard_map import shard_map
        import concourse.mybir as mybir
        from concourse import bass2jax

        bass2jax.install_neuronx_cc_hook()
        self.jax, self.jnp = jax, jnp

        partition_name = (nc.partition_id_tensor.name
                          if nc.partition_id_tensor else None)
        in_names, out_names, out_avals = [], [], []
        for alloc in nc.m.functions[0].allocations:
            if not isinstance(alloc, mybir.MemoryLocationSet):
                continue
            name = alloc.memorylocations[0].name
            if alloc.kind == "ExternalInput":
                if name != partition_name:
                    in_names.append(name)
            elif alloc.kind == "ExternalOutput":
                out_names.append(name)
                out_avals.append(jax.core.ShapedArray(
                    tuple(alloc.tensor_shape), mybir.dt.np(alloc.dtype)))
        assert nc.dbg_addr is None
        self.in_names, self.out_names, self.out_avals = in_names, out_names, out_avals
        n_params = len(in_names)
        all_in_names = in_names + out_names
        if partition_name is not None:
            all_in_names = all_in_names + [partition_name]
        all_in_names = tuple(all_in_names)

        def _body(*args):
            operands = list(args)
            if partition_name is not None:
                operands.append(bass2jax.partition_id_tensor())
            outs = bass2jax._bass_exec_p.bind(
                *operands,
                out_avals=tuple(out_avals),
                in_names=all_in_names,
                out_names=tuple(out_names),
                lowering_input_output_aliases=(),
                sim_require_finite=True,
                sim_require_nnan=True,
                nc=nc,
            )
            return tuple(outs)

        devices = jax.devices()[:N_CORES]
        self.mesh = Mesh(np.asarray(devices), ("core",))
        spec = PartitionSpec("core")
        n_outs = len(out_names)
        self.donate = tuple(range(n_params, n_params + n_outs))
        self.fn = jax.jit(
            shard_map(_body, mesh=self.mesh, in_specs=(spec,) * (n_params + n_outs),
                      out_specs=(spec,) * n_outs, check_rep=False),
            donate_argnums=self.donate, keep_unused=True)
        self.sharding = jax.sharding.NamedSharding(self.mesh, spec)

        def _zeros():
            return tuple(
                jnp.zeros((N_CORES * a.shape[0], *a.shape[1:]), a.dtype)
                for a in out_avals)
        self.zeros_fn = jax.jit(_zeros, out_shardings=(self.sharding,) * n_outs)

    def place_inputs(self, in_maps):
        concat = [np.concatenate([np.asarray(m[n]) for m in in_maps], axis=0)
                  for n in self.in_names]
        return [self.jax.device_put(a, self.sharding) for a in concat]

    def call(self, dev_in):
        outs = self.fn(*dev_in, *self.zeros_fn())
        self.jax.block_until_ready(outs)
        return outs

    def run(self, in_maps):
        outs = self.call(self.place_inputs(in_maps))
        res = []
        for c in range(N_CORES):
            res.append({n: np.asarray(outs[i]).reshape(
                N_CORES, *self.out_avals[i].shape)[c]
                for i, n in enumerate(self.out_names)})
        return res


def _get_runner(lh=LH, with_bias=False, null=False):
    key = (lh, with_bias, null)
    if key not in _NC_CACHE:
        nc = _build_null_nc(lh) if null else _build_nc(lh, with_bias)
        _NC_CACHE[key] = _Runner(nc)
    return _NC_CACHE[key]


def _bf16(a):
    import ml_dtypes
    return np.asarray(a, dtype=ml_dtypes.bfloat16)


def _make_eye32():
    return np.kron(np.eye(4, dtype=np.float32), np.ones((32, 32), np.float32))


def _make_in_maps(x, W_qkv, W_proj, b_proj, lh=LH):
    import ml_dtypes
    ncores_b = B * (L // lh)
    xb = np.ascontiguousarray(x.reshape(B, L // lh, lh, C))
    eye = _bf16(_make_eye32())
    ident = _bf16(np.eye(128, dtype=np.float32))
    w = _bf16(W_qkv)
    wp = _bf16(W_proj)
    bp = np.ascontiguousarray(b_proj, np.float32).reshape(1, C)
    in_maps = []
    for c in range(ncores_b):
        bb, hh = divmod(c, L // lh)
        xc = xb[bb, hh]                                   # [lh, C] f32
        xTc = _bf16(np.ascontiguousarray(xc.T))           # [C, lh]
        xNc = np.ones((lh, 258), dtype=ml_dtypes.bfloat16)
        xNc[:, 0:C] = _bf16(xc)
        in_maps.append({"xT": xTc, "xN": xNc, "w_qkv": w, "w_proj": wp,
                        "b_proj": bp, "eye32": eye, "id128": ident})
    return in_maps


def _assemble(results):
    outs = [results[c]["out"] for c in range(N_CORES)]
    y = np.stack(outs).reshape(B, 2, LH, C).reshape(B, L, C)
    return np.ascontiguousarray(y.reshape(BV, HW, C), dtype=np.float32)


def _run(x, W_qkv, W_proj, b_proj):
    with_bias = bool(np.any(b_proj))
    runner = _get_runner(LH, with_bias)
    in_maps = _make_in_maps(x, W_qkv, W_proj, b_proj)
    return _assemble(runner.run(in_maps))


def kernel(x, W_qkv, W_proj, b_proj):
    return _run(np.asarray(x, np.float32), np.asarray(W_qkv, np.float32),
                np.asarray(W_proj, np.float32), np.asarray(b_proj, np.float32))
